# revision 59
# baseline (speedup 1.0000x reference)
"""Trainium2 Bass kernel for a dense transformer block (B=128, T=256, C=384, H=6).

Sharding: data-parallel over batch across 8 NeuronCores (16 batches/core),
identical SPMD program per core, no collectives.

Design (v5, 371us):
  - per-core schedule: batches in pairs (free dim 512 in the big matmuls),
    software-pipelined: pair p's attention exp chain (ACT) drains behind
    pair p-1's MLP (PE).
  - DMA plan: few big triggers (sync-engine issue is ~600ns each); ALL
    input DMAs (x pair-merged, packed wq|wk|wv|wp, w1, w2) are issued
    before any output DMA so the sync queue never head-of-line blocks an
    input behind a store whose data isn't ready yet.
  - attention with TRANSPOSED scores sT[k,q] = kT^T @ qT: PSUM is
    pre-loaded with the causal mask bias (-30000 masked) via an identity
    matmul, K=64 score matmuls accumulate on top.  Each head pair shares
    one 2-bank PSUM tile, so a SINGLE strided ACT exp per (batch,
    head-pair) emits both masked attention-weight tiles directly in bf16
    - no separate mask multiply, no transpose-back, no diag trick.
  - softmax rowsums come for free from a ones-column appended to each
    head's v-slice ([d0..63|1.0] x 6 heads): apply-matmul column h*65+64
    accumulates sum_k w[k,q].  Normalization of all 6 heads is ONE DVE
    tensor_tensor with the per-head reciprocal broadcast along d via a
    stride-0 AP.
  - attn (token-major) -> C-major for the projection via identity-matmul
    transpose + one strided DVE copy per token tile.
  - K=64 score matmuls alternate between the two head PSUM banks so
    same-bank sub-array drains never collide (back-to-back K=64 matmuls
    into one bank crash the device).
  - relu split ACT/DVE per m1 chunk; LN stats/rstd on DVE, apply on ACT.
  - everything bf16 except the residual stream (x, y, out f32) and PSUM.
  - MLP down-projection i-outer so PSUM needs 1 rotating bank, not 4 held.
"""

import numpy as np

import concourse.bass as bass
import concourse.mybir as mybir
from concourse import bacc
from concourse.tile import TileContext
from contextlib import ExitStack

B, T, C = 128, 256, 384
H, D = 6, 64
FF = 4 * C
NCORES = 8
BL = B // NCORES  # 16
NPAIR = BL // 2  # 8
KC = C // 128  # 3
KH = FF // 128  # 12
EPS = 1e-5
F32 = mybir.dt.float32
F32R = mybir.dt.float32r
BF16 = mybir.dt.bfloat16
I32 = mybir.dt.int32
F8 = mybir.dt.float8e4
ALU = mybir.AluOpType
ACTF = mybir.ActivationFunctionType

EXP_S = float(2**23 / np.log(2.0))
EXP_B = float(127 * 2**23)
MASKB = 4.0e8  # masked scores -> it ~ 4e8 -> bitcast float ~1e-21 (safe to |s|<33)
SQRT_MAGIC = 0x1FBD1DF5
_STAGE = 99  # debug: truncate program after stage N (99 = full)


def build_program(use_g1, use_b1ln, use_g2, use_b2ln, use_bp, use_b1, use_b2):
    nc = bacc.Bacc(None)
    x = nc.declare_dram_parameter("x", [BL, T, C], F32, isOutput=False)
    # packed weights: [C, wq|wk|wv|wp] so one DMA trigger loads all four
    wqkvp = nc.declare_dram_parameter("wqkvp", [C, 4 * C], BF16, isOutput=False)
    w1 = nc.declare_dram_parameter("w1", [C, FF], BF16, isOutput=False)
    w2 = nc.declare_dram_parameter("w2", [FF, C], BF16, isOutput=False)
    g1 = nc.declare_dram_parameter("g1", [128, C], F32, isOutput=False)
    b1ln = nc.declare_dram_parameter("b1ln", [128, C], F32, isOutput=False)
    g2 = nc.declare_dram_parameter("g2", [128, C], F32, isOutput=False)
    b2ln = nc.declare_dram_parameter("b2ln", [128, C], F32, isOutput=False)
    bpb = nc.declare_dram_parameter("bpb", [128, C], F32, isOutput=False)
    b2b = nc.declare_dram_parameter("b2b", [128, C], F32, isOutput=False)
    b1c = nc.declare_dram_parameter("b1c", [128, KH], F32, isOutput=False)
    # packed constants: maskbias [0:384] | identb [384:512]
    consts = nc.declare_dram_parameter("consts", [128, 512], BF16, isOutput=False)
    out = nc.declare_dram_parameter("out", [BL, T, C], F32, isOutput=True)

    with TileContext(nc) as tc, ExitStack() as ctx:
        wts = ctx.enter_context(tc.tile_pool(name="wts", bufs=1))
        sb = ctx.enter_context(tc.tile_pool(name="sb", bufs=1))
        st = ctx.enter_context(tc.tile_pool(name="st", bufs=4))
        tr = ctx.enter_context(tc.tile_pool(name="tr", bufs=4))
        ps = ctx.enter_context(tc.tile_pool(name="ps", bufs=3, space="PSUM"))
        psS = ctx.enter_context(tc.tile_pool(name="psS", bufs=2, space="PSUM"))
        psy = ctx.enter_context(tc.tile_pool(name="psy", bufs=1, space="PSUM"))

        def load_one(dram, shape, tag, dt=F32):
            t_ = wts.tile(shape, dt, name=tag, tag=tag)
            nc.sync.dma_start(out=t_, in_=dram[:, :])
            return t_

        # ---- batched input DMA plan: few big triggers (sync-engine issue
        # is ~600ns each), ALL inputs issued before any output DMA so the
        # sync queue never head-of-line blocks an input behind a store ----
        consts_sb = load_one(consts, [128, 512], "consts", dt=BF16)
        maskb_sb = consts_sb[:, 0:384]
        id_bf = consts_sb[:, 384:512]

        xp_tiles = [None] * NPAIR

        def prefetch(p):
            xp = sb.tile([128, 4 * C], F32, name="xp", tag="xp", bufs=6)
            nc.sync.dma_start(
                out=xp.rearrange("p (b t c) -> p b t c", b=2, t=2),
                in_=x[2 * p : 2 * p + 2, :, :].rearrange(
                    "b (t q) c -> q b t c", q=128
                ),
            )
            xp_tiles[p] = xp
            return [xp[:, i * C : (i + 1) * C] for i in range(4)]

        xts = [None] * NPAIR
        xts[0] = prefetch(0)
        xts[1] = prefetch(1)

        # split the qkvp load in two triggers so wq/wk complete sooner
        wqkvp_sb = wts.tile([128, KC * 4 * C], BF16, name="wqkvp", tag="wqkvp")
        w3 = wqkvp_sb.rearrange("p (k f) -> p k f", k=KC)
        nc.sync.dma_start(
            out=w3[:, :, 0 : 2 * C],
            in_=wqkvp.rearrange("(k p) f -> p k f", p=128)[:, :, 0 : 2 * C],
        )
        wq_sb = [w3[:, k, 0:C] for k in range(KC)]
        wk_sb = [w3[:, k, C : 2 * C] for k in range(KC)]
        wv_sb = [w3[:, k, 2 * C : 3 * C] for k in range(KC)]
        wp_sb = [w3[:, k, 3 * C : 4 * C] for k in range(KC)]

        xts[2] = prefetch(2)

        nc.sync.dma_start(
            out=w3[:, :, 2 * C : 4 * C],
            in_=wqkvp.rearrange("(k p) f -> p k f", p=128)[:, :, 2 * C : 4 * C],
        )

        xts[3] = prefetch(3)

        w1t = wts.tile([128, KC * FF], BF16, name="w1t", tag="w1t")
        nc.sync.dma_start(
            out=w1t.rearrange("p (k f) -> p k f", k=KC),
            in_=w1.rearrange("(k p) f -> p k f", p=128),
        )
        w1_3 = w1t.rearrange("p (k f) -> p k f", k=KC)
        w1_sb = [w1_3[:, k, :] for k in range(KC)]

        w2t = wts.tile([128, KH * C], BF16, name="w2t", tag="w2t")
        nc.sync.dma_start(
            out=w2t.rearrange("p (m f) -> p m f", m=KH),
            in_=w2.rearrange("(m p) f -> p m f", p=128),
        )
        w2_3 = w2t.rearrange("p (m f) -> p m f", m=KH)
        w2_sb = [w2_3[:, m, :] for m in range(KH)]

        xts[4] = prefetch(4)
        xts[5] = prefetch(5)

        g1_sb = load_one(g1, [128, C], "g1") if use_g1 else None
        b1ln_sb = load_one(b1ln, [128, C], "b1ln") if use_b1ln else None
        g2_sb = load_one(g2, [128, C], "g2") if use_g2 else None
        b2ln_sb = load_one(b2ln, [128, C], "b2ln") if use_b2ln else None
        bpb_sb = load_one(bpb, [128, C], "bpb") if use_bp else None
        b2b_sb = load_one(b2b, [128, C], "b2b") if use_b2 else None
        b1c_sb = load_one(b1c, [128, KH], "b1c") if use_b1 else None

        def batched_rstd(mv8):
            """[128,8] interleaved (mean,var) x4 -> rstd4 [128,4]."""
            mv_v = mv8.rearrange("p (i two) -> p i two", two=2)
            var4 = mv_v[:, :, 1]
            vpe = st.tile([128, 4], F32, name="vpe", tag="vpe")
            nc.vector.tensor_scalar(
                out=vpe, in0=var4, scalar1=EPS, scalar2=None, op0=ALU.add)
            sd4 = st.tile([128, 4], F32, name="sd4", tag="sd4")
            nc.scalar.activation(sd4, vpe, ACTF.Sqrt)
            rstd4 = st.tile([128, 4], F32, name="rstd4", tag="rstd4")
            nc.vector.reciprocal(rstd4, sd4)
            return rstd4

        def ln_stat(mv8, i, src):
            stats = st.tile([128, 6], F32, name="lst", tag="lst")
            nc.vector.bn_stats(stats, src)
            nc.vector.bn_aggr(mv8[:, 2 * i : 2 * i + 2], stats)

        def layernorm4(dsts, srcs, g_sb, b_sb, mv8=None):
            if mv8 is None:
                mv8 = st.tile([128, 8], F32, name="mv8", tag="mv8")
                for i in range(4):
                    ln_stat(mv8, i, srcs[i])
            rstd4 = batched_rstd(mv8)
            mv_v2 = mv8.rearrange("p (i two) -> p i two", two=2)
            nmr4 = st.tile([128, 4], F32, name="nmr4", tag="nmr4")
            nc.vector.scalar_tensor_tensor(
                out=nmr4, in0=mv_v2[:, :, 0], scalar=-1.0, in1=rstd4,
                op0=ALU.mult, op1=ALU.mult,
            )
            for i in range(4):
                nc.scalar.activation(
                    dsts[i], srcs[i], ACTF.Identity,
                    bias=nmr4[:, i : i + 1], scale=rstd4[:, i : i + 1],
                )
                if g_sb is not None:
                    nc.vector.tensor_mul(dsts[i], dsts[i], g_sb)
                if b_sb is not None:
                    nc.vector.tensor_add(dsts[i], dsts[i], b_sb)

        def transpose4_into(dstT, srcs):
            """4x [128,C] token-major -> dstT [128, KC*2T] C-major packed."""
            dst3 = dstT.rearrange("q (c w) -> q c w", c=KC)
            for i in range(4):
                pt = ps.tile([128, C], F32, name="pa", tag="pa")
                for c in range(KC):
                    nc.tensor.matmul(
                        pt[:, c * 128 : (c + 1) * 128],
                        srcs[i][:, c * 128 : (c + 1) * 128],
                        id_bf,
                        start=True, stop=True,
                    )
                nc.scalar.copy(
                    dst3[:, :, i * 128 : (i + 1) * 128],
                    pt.rearrange("q (c w) -> q c w", c=KC),
                )


        def phase1a(p, xt):
            bs = [2 * p, 2 * p, 2 * p + 1, 2 * p + 1]
            tch = [0, 1, 0, 1]
            hT = sb.tile(
                [128, KC * 2 * T], BF16, name="hT", tag="hT", bufs=3
            )
            ht_ = [
                sb.tile([128, C], BF16, name=f"h{i}", tag=f"h{i}")
                for i in range(4)
            ]
            layernorm4(ht_, xt, g1_sb, b1ln_sb)
            transpose4_into(hT, ht_)

            # ---- stage 2: q^T (f32r), k^T (bf16) C-major; v token-major ----
            qT = [
                sb.tile([128, 2 * T], BF16, name=f"qT{m}", tag=f"qT{m}", bufs=3)
                for m in range(KC)
            ]
            kT = [
                sb.tile([128, 2 * T], BF16, name=f"kT{m}", tag=f"kT{m}", bufs=3)
                for m in range(KC)
            ]
            for m in range(KC):
                pq = ps.tile([128, 2 * T], F32, name="pa", tag="pa")
                for k in range(KC):
                    nc.tensor.matmul(
                        pq, wq_sb[k][:, m * 128 : (m + 1) * 128],
                        hT[:, k * 2 * T : (k + 1) * 2 * T],
                        start=(k == 0), stop=(k == KC - 1),
                    )
                nc.scalar.copy(qT[m], pq)
                pk = ps.tile([128, 2 * T], F32, name="pa", tag="pa")
                for k in range(KC):
                    nc.tensor.matmul(
                        pk, wk_sb[k][:, m * 128 : (m + 1) * 128],
                        hT[:, k * 2 * T : (k + 1) * 2 * T],
                        start=(k == 0), stop=(k == KC - 1),
                    )
                nc.vector.tensor_copy(kT[m], pk)
            # v token-major with a ones column per head: [h0 d0..63 1 | h1 ...]
            # col h*65+64 = 1.0 so the apply matmul emits rowsums for free.
            vx = [
                sb.tile([128, 6 * 65], BF16, name=f"vx{i}", tag=f"vx{i}",
                        bufs=3)
                for i in range(4)
            ]
            for i in range(4):
                pv = ps.tile([128, C], F32, name="pa", tag="pa")
                for k in range(KC):
                    nc.tensor.matmul(
                        pv, hT[:, k * 2 * T + i * 128 : k * 2 * T + (i + 1) * 128],
                        wv_sb[k],
                        start=(k == 0), stop=(k == KC - 1),
                    )
                vx3 = vx[i].rearrange("p (h w) -> p h w", w=65)
                nc.vector.tensor_copy(
                    vx3[:, :, 0:64], pv.rearrange("p (h w) -> p h w", w=64))
                nc.vector.memset(vx3[:, :, 64:65], 1.0)

            # ---- stage 3: attention, scores TRANSPOSED [k, q] ----
            # PSUM is pre-loaded with the causal mask bias (-30000 where
            # masked) via an identity matmul, then the K=64 score matmuls
            # accumulate on top; exp() then yields exact zeros for masked
            # slots, so the bf16 exp output IS the attention weight tile.
            # K=64 matmuls alternate between the two head tiles (different
            # PSUM banks) so same-bank sub-array drains never collide.
            ybs = {}
            for ib in range(2):
                tb = ib * T
                for j in range(KC):  # head pair (2j, 2j+1)
                    # one 2-bank tile: head 2j in bank 0 (cols 0:384),
                    # head 2j+1 in bank 1 (cols 512:896) -> ONE strided exp
                    pS2 = psS.tile([128, 1024], F32, name="psS", tag="psS")
                    pSa = pS2[:, 0:512]
                    pSb = pS2[:, 512:1024]
                    for pS in (pSa, pSb):
                        nc.tensor.matmul(
                            pS[:, 0:384], id_bf, maskb_sb,
                            start=True, stop=False, skip_group_check=True,
                        )
                    for pS, o in ((pSa, 0), (pSb, 64)):
                        # block k0: keys 0..127 x queries 0..255
                        nc.tensor.matmul(
                            pS[:, 0:256],
                            kT[j][o : o + 64, tb : tb + 128],
                            qT[j][o : o + 64, tb : tb + 256],
                            start=False, stop=True, skip_group_check=True,
                        )
                    for pS, o in ((pSa, 0), (pSb, 64)):
                        # block k1: keys 128..255 x queries 128..255
                        nc.tensor.matmul(
                            pS[:, 256:384],
                            kT[j][o : o + 64, tb + 128 : tb + 256],
                            qT[j][o : o + 64, tb + 128 : tb + 256],
                            start=False, stop=True, skip_group_check=True,
                        )
                    yb2 = tr.tile([128, 768], BF16, name=f"yb{ib}_{j}",
                                  tag=f"yb{ib}_{j}", bufs=1)
                    nc.scalar.activation(
                        yb2.rearrange("p (two f) -> p two f", two=2),
                        pS2.rearrange("p (two f) -> p two f", two=2)[
                            :, :, 0:384
                        ],
                        ACTF.Exp,
                    )
                    ybs[(ib, 2 * j)] = yb2[:, 0:384]
                    ybs[(ib, 2 * j + 1)] = yb2[:, 384:768]
            return dict(bs=bs, tch=tch, xt=xt, vx=vx, ybs=ybs)

        def phase1b(p, s):
            bs, tch, xt, vx = s["bs"], s["tch"], s["xt"], s["vx"]
            ybs = s["ybs"]
            acTa = sb.tile([128, KC * 2 * T], BF16, name="acTa", tag="acTa",
                           bufs=2)
            acT3 = acTa.rearrange("q (c w) -> q c w", c=KC)
            acT = [acTa[:, c * 2 * T : (c + 1) * 2 * T] for c in range(KC)]
            # apply: pA[q, h*65+d] = sum_k ybT[k, q] * vx[k, h*65+d];
            # column h*65+64 accumulates the softmax rowsum (ones column).
            for ib in range(2):
                for tc in range(2):
                    i = ib * 2 + tc
                    pA = ps.tile([128, 512], F32, name="pa", tag="pa")
                    for hh in range(6):
                        yb = ybs[(ib, hh)]
                        sl = slice(hh * 65, hh * 65 + 65)
                        if tc == 0:
                            nc.tensor.matmul(
                                pA[:, sl], yb[:, 0:128],
                                vx[ib * 2][:, sl],
                                start=True, stop=True,
                            )
                        else:
                            nc.tensor.matmul(
                                pA[:, sl], yb[:, 128:256],
                                vx[ib * 2][:, sl],
                                start=True, stop=False,
                            )
                            nc.tensor.matmul(
                                pA[:, sl], yb[:, 256:384],
                                vx[ib * 2 + 1][:, sl],
                                start=False, stop=True,
                            )
                    pA3 = pA[:, 0 : 6 * 65].rearrange("p (h w) -> p h w", w=65)
                    rr6 = st.tile([128, 6], F32, name="rr6", tag="rr6")
                    nc.vector.reciprocal(rr6, pA3[:, :, 64])
                    at = sb.tile([128, C], BF16, name=f"at{i}",
                                 tag=f"at{i}", bufs=2)
                    # normalize all 6 heads in ONE DVE op: per-head scale
                    # broadcast along d via a stride-0 AP
                    nc.vector.tensor_tensor(
                        out=at.rearrange("p (h d) -> p h d", h=6),
                        in0=pA3[:, :, 0:64],
                        in1=rr6.rearrange("p (h one) -> p h one", one=1)
                        .broadcast_to((128, 6, 64)),
                        op=ALU.mult,
                    )
                    # transpose attn to C-major for the projection
                    pt = ps.tile([128, C], F32, name="pa", tag="pa")
                    for c in range(KC):
                        nc.tensor.matmul(
                            pt[:, c * 128 : (c + 1) * 128],
                            at[:, c * 128 : (c + 1) * 128],
                            id_bf,
                            start=True, stop=True,
                        )
                    nc.vector.tensor_copy(
                        acT3[:, :, i * 128 : (i + 1) * 128],
                        pt.rearrange("q (c w) -> q c w", c=KC),
                    )

            # ---- stage 4: proj + residual -> y ----
            yt = [
                sb.tile([128, C], F32, name=f"y{i}", tag=f"y{i}", bufs=2)
                for i in range(4)
            ]
            mv8b = st.tile([128, 8], F32, name="mv8", tag="mv8")
            for i in range(4):
                pP = ps.tile([128, C], F32, name="pa", tag="pa")
                for k in range(KC):
                    nc.tensor.matmul(
                        pP, acT[k][:, i * 128 : (i + 1) * 128], wp_sb[k],
                        start=(k == 0), stop=(k == KC - 1),
                    )
                nc.vector.tensor_add(yt[i], pP, xt[i])
                if bpb_sb is not None:
                    nc.vector.tensor_add(yt[i], yt[i], bpb_sb)
                ln_stat(mv8b, i, yt[i])

            h2_ = [
                sb.tile([128, C], BF16, name=f"h2{i}", tag=f"h2{i}", bufs=2)
                for i in range(4)
            ]
            layernorm4(h2_, yt, g2_sb, b2ln_sb, mv8=mv8b)
            return dict(bs=bs, tch=tch, yt=yt, h2_=h2_)

        def phase2(s):
            bs, tch, yt, h2_ = s["bs"], s["tch"], s["yt"], s["h2_"]
            h2T = sb.tile(
                [128, KC * 2 * T], BF16, name="h2T", tag="h2T", bufs=2
            )
            transpose4_into(h2T, h2_)
            # ---- stage 6: MLP up + relu ----
            m1r = sb.tile([128, KH * 2 * T], BF16, name="m1r", tag="m1r")
            m1r3 = m1r.rearrange("p (m n) -> p m n", m=KH)
            for m in range(KH):
                pM = ps.tile([128, 2 * T], F32, name="pa", tag="pa")
                for k in range(KC):
                    nc.tensor.matmul(
                        pM, w1_sb[k][:, m * 128 : (m + 1) * 128],
                        h2T[:, k * 2 * T : (k + 1) * 2 * T],
                        start=(k == 0), stop=(k == KC - 1),
                    )
                if m % 2 == 0:
                    nc.scalar.activation(
                        m1r3[:, m, :], pM, ACTF.Relu,
                        bias=(b1c_sb[:, m : m + 1] if use_b1 else 0.0),
                    )
                else:
                    nc.vector.tensor_scalar(
                        out=m1r3[:, m, :], in0=pM,
                        scalar1=(b1c_sb[:, m : m + 1] if use_b1 else 0.0),
                        scalar2=0.0, op0=ALU.add, op1=ALU.max,
                    )

            # ---- stage 7: MLP down (i-outer) + residual + store ----
            for i in range(4):
                # padded to 512 so each psY slot is bank-aligned (mm out
                # must not cross a 2KB PSUM bank)
                pY = psy.tile([128, 512], F32, name="psY", tag="psY")
                for m in range(KH):
                    nc.tensor.matmul(
                        pY[:, 0:C],
                        m1r3[:, m, i * 128 : (i + 1) * 128], w2_sb[m],
                        start=(m == 0), stop=(m == KH - 1),
                    )
                ot = sb.tile([128, C], F32, name=f"ot{i}", tag=f"ot{i}", bufs=2)
                nc.vector.tensor_add(ot, pY[:, 0:C], yt[i])
                if b2b_sb is not None:
                    nc.vector.tensor_add(ot, ot, b2b_sb)
                nc.sync.dma_start(
                    out=out[bs[i], tch[i] * 128 : (tch[i] + 1) * 128, :],
                    in_=ot,
                )

        # Staggered schedule: pair p's exp chains (DVE/ACT) drain behind
        # pair p-1's MLP (PE); attention pass B then finds its inputs ready.
        sa = [None] * NPAIR
        sb_ = [None] * NPAIR
        sa[0] = phase1a(0, xts[0])
        sb_[0] = phase1b(0, sa[0])
        for p in range(1, NPAIR):
            if p + 5 < NPAIR:
                xts[p + 5] = prefetch(p + 5)
            sa[p] = phase1a(p, xts[p])
            # pair p-1's MLP streams on the PE while pair p's exp chains
            # drain on ACT, so 1b(p)'s apply matmuls find their stationary
            # attention-weight tiles ready
            phase2(sb_[p - 1])
            sb_[p] = phase1b(p, sa[p])
        phase2(sb_[NPAIR - 1])

    nc.compile()
    return nc


def _host_prep(inputs):
    f = np.float32
    x = np.ascontiguousarray(inputs["x"], dtype=f)
    import ml_dtypes as _md

    _bf = _md.bfloat16
    wq_full = np.ascontiguousarray(
        (np.asarray(inputs["wq"], dtype=f).transpose(1, 0, 2).reshape(C, C)
         * (C ** -0.5)).astype(_bf)
    )
    wk_full = np.ascontiguousarray(
        np.asarray(inputs["wk"], dtype=f).transpose(1, 0, 2)
        .reshape(C, C).astype(_bf)
    )
    wv_full = np.ascontiguousarray(
        np.asarray(inputs["wv"], dtype=f).transpose(1, 0, 2)
        .reshape(C, C).astype(_bf)
    )
    import ml_dtypes

    bf = ml_dtypes.bfloat16
    wp = np.ascontiguousarray(np.asarray(inputs["w_proj"], dtype=f).astype(bf))
    wqkvp = np.ascontiguousarray(
        np.concatenate([wq_full, wk_full, wv_full, wp], axis=1))
    w1 = np.ascontiguousarray(np.asarray(inputs["w1"], dtype=f).astype(bf))
    w2 = np.ascontiguousarray(np.asarray(inputs["w2"], dtype=f).astype(bf))
    tile128 = lambda v: np.ascontiguousarray(
        np.broadcast_to(np.asarray(v, dtype=f), (128, C))
    )
    g1 = tile128(inputs["ln1_g"])
    b1ln = tile128(inputs["ln1_b"])
    g2 = tile128(inputs["ln2_g"])
    b2ln = tile128(inputs["ln2_b"])
    bpb = tile128(inputs["b_proj"])
    b2b = tile128(inputs["b2"])
    b1c = np.ascontiguousarray(
        np.asarray(inputs["b1"], dtype=f).reshape(KH, 128).T)
    # transposed-scores mask bias [k, q]: 0 where valid (k<=q), -30000
    # where masked; exp(s - 30000) == 0 exactly in bf16.
    triu = np.triu(np.ones((128, 128), dtype=np.float32))
    maskT = np.concatenate(
        [triu, np.ones((128, 128), dtype=np.float32), triu], axis=1)
    maskbias = (maskT - 1.0) * 30000.0
    identb = np.eye(128, dtype=f)
    consts = np.ascontiguousarray(
        np.concatenate([maskbias, identb], axis=1).astype(bf))

    flags = (
        bool(not np.all(np.asarray(inputs["ln1_g"]) == 1.0)),
        bool(np.any(np.asarray(inputs["ln1_b"]))),
        bool(not np.all(np.asarray(inputs["ln2_g"]) == 1.0)),
        bool(np.any(np.asarray(inputs["ln2_b"]))),
        bool(np.any(np.asarray(inputs["b_proj"]))),
        bool(np.any(np.asarray(inputs["b1"]))),
        bool(np.any(np.asarray(inputs["b2"]))),
    )
    shared = dict(
        wqkvp=wqkvp, w1=w1, w2=w2,
        g1=g1, b1ln=b1ln, g2=g2, b2ln=b2ln, bpb=bpb, b2b=b2b, b1c=b1c,
        consts=consts,
    )
    in_maps = []
    for i in range(NCORES):
        m = dict(shared)
        m["x"] = np.ascontiguousarray(x[i * BL : (i + 1) * BL])
        in_maps.append(m)
    return in_maps, flags


_NC_CACHE = {}


def _get_program(flags):
    key = (flags, _STAGE)
    if key not in _NC_CACHE:
        _NC_CACHE[key] = build_program(*flags)
    return _NC_CACHE[key]


def run(inputs, **spmd_kwargs):
    from concourse.bass_utils import run_bass_kernel_spmd

    in_maps, flags = _host_prep(inputs)
    nc = _get_program(flags)
    bkr = run_bass_kernel_spmd(nc, in_maps, list(range(NCORES)), **spmd_kwargs)
    outs = [bkr.results[i]["out"] for i in range(NCORES)]
    return np.concatenate(outs, axis=0).astype(np.float32), bkr


def kernel(**inputs):
    full, _ = run(inputs)
    return full



# revision 60
# speedup vs baseline: 1.1748x; 1.1748x over previous
"""Trainium2 Bass kernel for a dense transformer block (B=128, T=256, C=384, H=6).

Sharding: data-parallel over batch across 8 NeuronCores (16 batches/core),
identical SPMD program per core, no collectives.

Design (v4, 394us):
  - per-core schedule: batches in pairs (free dim 512 in the big matmuls),
    software-pipelined: pair p's attention exp chain (ACT) drains behind
    pair p-1's MLP (PE).
  - DMA plan: few big triggers (sync-engine issue is ~600ns each); ALL
    input DMAs (x pair-merged, packed wq|wk|wv|wp, w1, w2) are issued
    before any output DMA so the sync queue never head-of-line blocks an
    input behind a store whose data isn't ready yet.
  - attention with TRANSPOSED scores sT[k,q] = kT^T @ qT: PSUM is
    pre-loaded with the causal mask bias (-30000 masked) via an identity
    matmul, K=64 score matmuls accumulate on top, and a single ACT exp per
    (batch, head) then emits the masked attention-weight tile directly in
    bf16 - no separate mask multiply, no transpose-back, no diag trick.
  - softmax rowsums come for free from a ones-column appended to each
    head's v-slice ([d0..63|1.0] x 6 heads): apply-matmul column h*65+64
    accumulates sum_k w[k,q].  Normalization is then a per-partition
    scale (ACT Identity-scale / DVE tensor_scalar alternating per head).
  - attn (token-major) -> C-major for the projection via identity-matmul
    transpose + one strided DVE copy per token tile.
  - K=64 score matmuls alternate between the two head PSUM tiles so
    same-bank sub-array drains never collide (back-to-back K=64 matmuls
    into one bank crash the device).
  - relu split ACT/DVE per m1 chunk; LN stats/rstd on DVE, apply on ACT.
  - everything bf16 except the residual stream (x, y, out f32) and PSUM.
  - MLP down-projection i-outer so PSUM needs 1 rotating bank, not 4 held.
"""

import numpy as np

import concourse.bass as bass
import concourse.mybir as mybir
from concourse import bacc
from concourse.tile import TileContext
from contextlib import ExitStack

B, T, C = 128, 256, 384
H, D = 6, 64
FF = 4 * C
NCORES = 8
BL = B // NCORES  # 16
NPAIR = BL // 2  # 8
KC = C // 128  # 3
KH = FF // 128  # 12
EPS = 1e-5
F32 = mybir.dt.float32
F32R = mybir.dt.float32r
BF16 = mybir.dt.bfloat16
I32 = mybir.dt.int32
F8 = mybir.dt.float8e4
ALU = mybir.AluOpType
ACTF = mybir.ActivationFunctionType

EXP_S = float(2**23 / np.log(2.0))
EXP_B = float(127 * 2**23)
MASKB = 4.0e8  # masked scores -> it ~ 4e8 -> bitcast float ~1e-21 (safe to |s|<33)
SQRT_MAGIC = 0x1FBD1DF5
_STAGE = 99  # debug: truncate program after stage N (99 = full)


def build_program(use_g1, use_b1ln, use_g2, use_b2ln, use_bp, use_b1, use_b2):
    nc = bacc.Bacc(None)
    x = nc.declare_dram_parameter("x", [BL, T, C], BF16, isOutput=False)
    # packed weights: [C, wq|wk|wv|wp] so one DMA trigger loads all four
    wqkvp = nc.declare_dram_parameter("wqkvp", [C, 4 * C], BF16, isOutput=False)
    w1 = nc.declare_dram_parameter("w1", [C, FF], BF16, isOutput=False)
    w2 = nc.declare_dram_parameter("w2", [FF, C], BF16, isOutput=False)
    g1 = nc.declare_dram_parameter("g1", [128, C], F32, isOutput=False)
    b1ln = nc.declare_dram_parameter("b1ln", [128, C], F32, isOutput=False)
    g2 = nc.declare_dram_parameter("g2", [128, C], F32, isOutput=False)
    b2ln = nc.declare_dram_parameter("b2ln", [128, C], F32, isOutput=False)
    bpb = nc.declare_dram_parameter("bpb", [128, C], F32, isOutput=False)
    b2b = nc.declare_dram_parameter("b2b", [128, C], F32, isOutput=False)
    b1c = nc.declare_dram_parameter("b1c", [128, KH], F32, isOutput=False)
    # packed constants: maskbias [0:384] | identb [384:512]
    consts = nc.declare_dram_parameter("consts", [128, 512], BF16, isOutput=False)
    out = nc.declare_dram_parameter("out", [BL, T, C], F32, isOutput=True)

    with TileContext(nc) as tc, ExitStack() as ctx:
        wts = ctx.enter_context(tc.tile_pool(name="wts", bufs=1))
        sb = ctx.enter_context(tc.tile_pool(name="sb", bufs=1))
        st = ctx.enter_context(tc.tile_pool(name="st", bufs=4))
        tr = ctx.enter_context(tc.tile_pool(name="tr", bufs=4))
        ps = ctx.enter_context(tc.tile_pool(name="ps", bufs=3, space="PSUM"))
        psS = ctx.enter_context(tc.tile_pool(name="psS", bufs=2, space="PSUM"))
        psy = ctx.enter_context(tc.tile_pool(name="psy", bufs=1, space="PSUM"))

        def load_one(dram, shape, tag, dt=F32):
            t_ = wts.tile(shape, dt, name=tag, tag=tag)
            nc.sync.dma_start(out=t_, in_=dram[:, :])
            return t_

        # ---- batched input DMA plan: few big triggers (sync-engine issue
        # is ~600ns each), ALL inputs issued before any output DMA so the
        # sync queue never head-of-line blocks an input behind a store ----
        consts_sb = load_one(consts, [128, 512], "consts", dt=BF16)
        maskb_sb = consts_sb[:, 0:384]
        id_bf = consts_sb[:, 384:512]

        xp_tiles = [None] * NPAIR

        def prefetch(p):
            xp = sb.tile([128, 4 * C], BF16, name="xp", tag="xp", bufs=6)
            nc.sync.dma_start(
                out=xp.rearrange("p (b t c) -> p b t c", b=2, t=2),
                in_=x[2 * p : 2 * p + 2, :, :].rearrange(
                    "b (t q) c -> q b t c", q=128
                ),
            )
            xp_tiles[p] = xp
            return [xp[:, i * C : (i + 1) * C] for i in range(4)]

        xts = [None] * NPAIR
        xts[0] = prefetch(0)
        xts[1] = prefetch(1)

        # split the qkvp load in two triggers so wq/wk complete sooner
        wqkvp_sb = wts.tile([128, KC * 4 * C], BF16, name="wqkvp", tag="wqkvp")
        w3 = wqkvp_sb.rearrange("p (k f) -> p k f", k=KC)
        nc.sync.dma_start(
            out=w3[:, :, 0 : 2 * C],
            in_=wqkvp.rearrange("(k p) f -> p k f", p=128)[:, :, 0 : 2 * C],
        )
        wq_sb = [w3[:, k, 0:C] for k in range(KC)]
        wk_sb = [w3[:, k, C : 2 * C] for k in range(KC)]
        wv_sb = [w3[:, k, 2 * C : 3 * C] for k in range(KC)]
        wp_sb = [w3[:, k, 3 * C : 4 * C] for k in range(KC)]

        xts[2] = prefetch(2)

        nc.sync.dma_start(
            out=w3[:, :, 2 * C : 4 * C],
            in_=wqkvp.rearrange("(k p) f -> p k f", p=128)[:, :, 2 * C : 4 * C],
        )

        xts[3] = prefetch(3)

        w1t = wts.tile([128, KC * FF], BF16, name="w1t", tag="w1t")
        nc.sync.dma_start(
            out=w1t.rearrange("p (k f) -> p k f", k=KC),
            in_=w1.rearrange("(k p) f -> p k f", p=128),
        )
        w1_3 = w1t.rearrange("p (k f) -> p k f", k=KC)
        w1_sb = [w1_3[:, k, :] for k in range(KC)]

        w2t = wts.tile([128, KH * C], BF16, name="w2t", tag="w2t")
        nc.sync.dma_start(
            out=w2t.rearrange("p (m f) -> p m f", m=KH),
            in_=w2.rearrange("(m p) f -> p m f", p=128),
        )
        w2_3 = w2t.rearrange("p (m f) -> p m f", m=KH)
        w2_sb = [w2_3[:, m, :] for m in range(KH)]

        xts[4] = prefetch(4)
        xts[5] = prefetch(5)

        g1_sb = load_one(g1, [128, C], "g1") if use_g1 else None
        b1ln_sb = load_one(b1ln, [128, C], "b1ln") if use_b1ln else None
        g2_sb = load_one(g2, [128, C], "g2") if use_g2 else None
        b2ln_sb = load_one(b2ln, [128, C], "b2ln") if use_b2ln else None
        bpb_sb = load_one(bpb, [128, C], "bpb") if use_bp else None
        b2b_sb = load_one(b2b, [128, C], "b2b") if use_b2 else None
        b1c_sb = load_one(b1c, [128, KH], "b1c") if use_b1 else None

        def batched_rstd(mv8):
            """[128,8] interleaved (mean,var) x4 -> rstd4 [128,4]."""
            mv_v = mv8.rearrange("p (i two) -> p i two", two=2)
            var4 = mv_v[:, :, 1]
            vpe = st.tile([128, 4], F32, name="vpe", tag="vpe")
            nc.vector.tensor_scalar(
                out=vpe, in0=var4, scalar1=EPS, scalar2=None, op0=ALU.add)
            sd4 = st.tile([128, 4], F32, name="sd4", tag="sd4")
            nc.scalar.activation(sd4, vpe, ACTF.Sqrt)
            rstd4 = st.tile([128, 4], F32, name="rstd4", tag="rstd4")
            nc.vector.reciprocal(rstd4, sd4)
            return rstd4

        def ln_stat(mv8, i, src):
            stats = st.tile([128, 6], F32, name="lst", tag="lst")
            nc.vector.bn_stats(stats, src)
            nc.vector.bn_aggr(mv8[:, 2 * i : 2 * i + 2], stats)

        def layernorm4(dsts, srcs, g_sb, b_sb, mv8=None):
            if mv8 is None:
                mv8 = st.tile([128, 8], F32, name="mv8", tag="mv8")
                for i in range(4):
                    ln_stat(mv8, i, srcs[i])
            rstd4 = batched_rstd(mv8)
            mv_v2 = mv8.rearrange("p (i two) -> p i two", two=2)
            nmr4 = st.tile([128, 4], F32, name="nmr4", tag="nmr4")
            nc.vector.scalar_tensor_tensor(
                out=nmr4, in0=mv_v2[:, :, 0], scalar=-1.0, in1=rstd4,
                op0=ALU.mult, op1=ALU.mult,
            )
            for i in range(4):
                nc.scalar.activation(
                    dsts[i], srcs[i], ACTF.Identity,
                    bias=nmr4[:, i : i + 1], scale=rstd4[:, i : i + 1],
                )
                if g_sb is not None:
                    nc.vector.tensor_mul(dsts[i], dsts[i], g_sb)
                if b_sb is not None:
                    nc.vector.tensor_add(dsts[i], dsts[i], b_sb)

        def transpose4_into(dstT, srcs):
            """4x [128,C] token-major -> dstT [128, KC*2T] C-major packed."""
            dst3 = dstT.rearrange("q (c w) -> q c w", c=KC)
            for i in range(4):
                pt = ps.tile([128, C], F32, name="pa", tag="pa")
                for c in range(KC):
                    nc.tensor.matmul(
                        pt[:, c * 128 : (c + 1) * 128],
                        srcs[i][:, c * 128 : (c + 1) * 128],
                        id_bf,
                        start=True, stop=True,
                    )
                nc.scalar.copy(
                    dst3[:, :, i * 128 : (i + 1) * 128],
                    pt.rearrange("q (c w) -> q c w", c=KC),
                )


        def phase1a(p, xt):
            bs = [2 * p, 2 * p, 2 * p + 1, 2 * p + 1]
            tch = [0, 1, 0, 1]
            hT = sb.tile(
                [128, KC * 2 * T], BF16, name="hT", tag="hT", bufs=3
            )
            ht_ = [
                sb.tile([128, C], BF16, name=f"h{i}", tag=f"h{i}")
                for i in range(4)
            ]
            layernorm4(ht_, xt, g1_sb, b1ln_sb)
            transpose4_into(hT, ht_)

            # ---- stage 2: q^T (f32r), k^T (bf16) C-major; v token-major ----
            qT = [
                sb.tile([128, 2 * T], BF16, name=f"qT{m}", tag=f"qT{m}", bufs=3)
                for m in range(KC)
            ]
            kT = [
                sb.tile([128, 2 * T], BF16, name=f"kT{m}", tag=f"kT{m}", bufs=3)
                for m in range(KC)
            ]
            for m in range(KC):
                pq = ps.tile([128, 2 * T], F32, name="pa", tag="pa")
                for k in range(KC):
                    nc.tensor.matmul(
                        pq, wq_sb[k][:, m * 128 : (m + 1) * 128],
                        hT[:, k * 2 * T : (k + 1) * 2 * T],
                        start=(k == 0), stop=(k == KC - 1),
                    )
                nc.scalar.copy(qT[m], pq)
                pk = ps.tile([128, 2 * T], F32, name="pa", tag="pa")
                for k in range(KC):
                    nc.tensor.matmul(
                        pk, wk_sb[k][:, m * 128 : (m + 1) * 128],
                        hT[:, k * 2 * T : (k + 1) * 2 * T],
                        start=(k == 0), stop=(k == KC - 1),
                    )
                nc.vector.tensor_copy(kT[m], pk)
            # v token-major with a ones column per head: [h0 d0..63 1 | h1 ...]
            # col h*65+64 = 1.0 so the apply matmul emits rowsums for free.
            vx = [
                sb.tile([128, 6 * 65], BF16, name=f"vx{i}", tag=f"vx{i}",
                        bufs=3)
                for i in range(4)
            ]
            for i in range(4):
                pv = ps.tile([128, C], F32, name="pa", tag="pa")
                for k in range(KC):
                    nc.tensor.matmul(
                        pv, hT[:, k * 2 * T + i * 128 : k * 2 * T + (i + 1) * 128],
                        wv_sb[k],
                        start=(k == 0), stop=(k == KC - 1),
                    )
                vx3 = vx[i].rearrange("p (h w) -> p h w", w=65)
                nc.vector.tensor_copy(
                    vx3[:, :, 0:64], pv.rearrange("p (h w) -> p h w", w=64))
                nc.vector.memset(vx3[:, :, 64:65], 1.0)

            # ---- stage 3: attention, scores TRANSPOSED [k, q] ----
            # PSUM is pre-loaded with the causal mask bias (-30000 where
            # masked) via an identity matmul, then the K=64 score matmuls
            # accumulate on top; exp() then yields exact zeros for masked
            # slots, so the bf16 exp output IS the attention weight tile.
            # K=64 matmuls alternate between the two head tiles (different
            # PSUM banks) so same-bank sub-array drains never collide.
            ybs = {}
            for ib in range(2):
                tb = ib * T
                for j in range(KC):  # head pair (2j, 2j+1)
                    # one 2-bank tile: head 2j in bank 0 (cols 0:384),
                    # head 2j+1 in bank 1 (cols 512:896) -> ONE strided exp
                    pS2 = psS.tile([128, 1024], F32, name="psS", tag="psS")
                    pSa = pS2[:, 0:512]
                    pSb = pS2[:, 512:1024]
                    for pS in (pSa, pSb):
                        nc.tensor.matmul(
                            pS[:, 0:384], id_bf, maskb_sb,
                            start=True, stop=False, skip_group_check=True,
                        )
                    for pS, o in ((pSa, 0), (pSb, 64)):
                        # block k0: keys 0..127 x queries 0..255
                        nc.tensor.matmul(
                            pS[:, 0:256],
                            kT[j][o : o + 64, tb : tb + 128],
                            qT[j][o : o + 64, tb : tb + 256],
                            start=False, stop=True, skip_group_check=True,
                        )
                    for pS, o in ((pSa, 0), (pSb, 64)):
                        # block k1: keys 128..255 x queries 128..255
                        nc.tensor.matmul(
                            pS[:, 256:384],
                            kT[j][o : o + 64, tb + 128 : tb + 256],
                            qT[j][o : o + 64, tb + 128 : tb + 256],
                            start=False, stop=True, skip_group_check=True,
                        )
                    yb2 = tr.tile([128, 768], BF16, name=f"yb{ib}_{j}",
                                  tag=f"yb{ib}_{j}", bufs=1)
                    nc.scalar.activation(
                        yb2.rearrange("p (two f) -> p two f", two=2),
                        pS2.rearrange("p (two f) -> p two f", two=2)[
                            :, :, 0:384
                        ],
                        ACTF.Exp,
                    )
                    ybs[(ib, 2 * j)] = yb2[:, 0:384]
                    ybs[(ib, 2 * j + 1)] = yb2[:, 384:768]
            return dict(bs=bs, tch=tch, xt=xt, vx=vx, ybs=ybs)

        def phase1b(p, s):
            bs, tch, xt, vx = s["bs"], s["tch"], s["xt"], s["vx"]
            ybs = s["ybs"]
            acTa = sb.tile([128, KC * 2 * T], BF16, name="acTa", tag="acTa",
                           bufs=2)
            acT3 = acTa.rearrange("q (c w) -> q c w", c=KC)
            acT = [acTa[:, c * 2 * T : (c + 1) * 2 * T] for c in range(KC)]
            # apply: pA[q, h*65+d] = sum_k ybT[k, q] * vx[k, h*65+d];
            # column h*65+64 accumulates the softmax rowsum (ones column).
            for ib in range(2):
                for tc in range(2):
                    i = ib * 2 + tc
                    pA = ps.tile([128, 512], F32, name="pa", tag="pa")
                    for hh in range(6):
                        yb = ybs[(ib, hh)]
                        sl = slice(hh * 65, hh * 65 + 65)
                        if tc == 0:
                            nc.tensor.matmul(
                                pA[:, sl], yb[:, 0:128],
                                vx[ib * 2][:, sl],
                                start=True, stop=True,
                            )
                        else:
                            nc.tensor.matmul(
                                pA[:, sl], yb[:, 128:256],
                                vx[ib * 2][:, sl],
                                start=True, stop=False,
                            )
                            nc.tensor.matmul(
                                pA[:, sl], yb[:, 256:384],
                                vx[ib * 2 + 1][:, sl],
                                start=False, stop=True,
                            )
                    pA3 = pA[:, 0 : 6 * 65].rearrange("p (h w) -> p h w", w=65)
                    rr6 = st.tile([128, 6], F32, name="rr6", tag="rr6")
                    nc.vector.reciprocal(rr6, pA3[:, :, 64])
                    at = sb.tile([128, C], BF16, name=f"at{i}",
                                 tag=f"at{i}", bufs=2)
                    # normalize all 6 heads in ONE DVE op: per-head scale
                    # broadcast along d via a stride-0 AP
                    nc.vector.tensor_tensor(
                        out=at.rearrange("p (h d) -> p h d", h=6),
                        in0=pA3[:, :, 0:64],
                        in1=rr6.rearrange("p (h one) -> p h one", one=1)
                        .broadcast_to((128, 6, 64)),
                        op=ALU.mult,
                    )
                    # transpose attn to C-major for the projection
                    pt = ps.tile([128, C], F32, name="pa", tag="pa")
                    for c in range(KC):
                        nc.tensor.matmul(
                            pt[:, c * 128 : (c + 1) * 128],
                            at[:, c * 128 : (c + 1) * 128],
                            id_bf,
                            start=True, stop=True,
                        )
                    nc.vector.tensor_copy(
                        acT3[:, :, i * 128 : (i + 1) * 128],
                        pt.rearrange("q (c w) -> q c w", c=KC),
                    )

            # ---- stage 4: proj + residual -> y ----
            yt = [
                sb.tile([128, C], F32, name=f"y{i}", tag=f"y{i}", bufs=2)
                for i in range(4)
            ]
            mv8b = st.tile([128, 8], F32, name="mv8", tag="mv8")
            for i in range(4):
                pP = ps.tile([128, C], F32, name="pa", tag="pa")
                for k in range(KC):
                    nc.tensor.matmul(
                        pP, acT[k][:, i * 128 : (i + 1) * 128], wp_sb[k],
                        start=(k == 0), stop=(k == KC - 1),
                    )
                nc.vector.tensor_add(yt[i], pP, xt[i])
                if bpb_sb is not None:
                    nc.vector.tensor_add(yt[i], yt[i], bpb_sb)
                ln_stat(mv8b, i, yt[i])

            h2_ = [
                sb.tile([128, C], BF16, name=f"h2{i}", tag=f"h2{i}", bufs=2)
                for i in range(4)
            ]
            layernorm4(h2_, yt, g2_sb, b2ln_sb, mv8=mv8b)
            return dict(bs=bs, tch=tch, yt=yt, h2_=h2_)

        def phase2(s):
            bs, tch, yt, h2_ = s["bs"], s["tch"], s["yt"], s["h2_"]
            h2T = sb.tile(
                [128, KC * 2 * T], BF16, name="h2T", tag="h2T", bufs=2
            )
            transpose4_into(h2T, h2_)
            # ---- stage 6: MLP up + relu ----
            m1r = sb.tile([128, KH * 2 * T], BF16, name="m1r", tag="m1r")
            m1r3 = m1r.rearrange("p (m n) -> p m n", m=KH)
            for m in range(KH):
                pM = ps.tile([128, 2 * T], F32, name="pa", tag="pa")
                for k in range(KC):
                    nc.tensor.matmul(
                        pM, w1_sb[k][:, m * 128 : (m + 1) * 128],
                        h2T[:, k * 2 * T : (k + 1) * 2 * T],
                        start=(k == 0), stop=(k == KC - 1),
                    )
                if m % 2 == 0:
                    nc.scalar.activation(
                        m1r3[:, m, :], pM, ACTF.Relu,
                        bias=(b1c_sb[:, m : m + 1] if use_b1 else 0.0),
                    )
                else:
                    nc.vector.tensor_scalar(
                        out=m1r3[:, m, :], in0=pM,
                        scalar1=(b1c_sb[:, m : m + 1] if use_b1 else 0.0),
                        scalar2=0.0, op0=ALU.add, op1=ALU.max,
                    )

            # ---- stage 7: MLP down (i-outer) + residual + store ----
            for i in range(4):
                # padded to 512 so each psY slot is bank-aligned (mm out
                # must not cross a 2KB PSUM bank)
                pY = psy.tile([128, 512], F32, name="psY", tag="psY")
                for m in range(KH):
                    nc.tensor.matmul(
                        pY[:, 0:C],
                        m1r3[:, m, i * 128 : (i + 1) * 128], w2_sb[m],
                        start=(m == 0), stop=(m == KH - 1),
                    )
                ot = sb.tile([128, C], F32, name=f"ot{i}", tag=f"ot{i}", bufs=2)
                nc.vector.tensor_add(ot, pY[:, 0:C], yt[i])
                if b2b_sb is not None:
                    nc.vector.tensor_add(ot, ot, b2b_sb)
                nc.sync.dma_start(
                    out=out[bs[i], tch[i] * 128 : (tch[i] + 1) * 128, :],
                    in_=ot,
                )

        # Staggered schedule: pair p's exp chains (DVE/ACT) drain behind
        # pair p-1's MLP (PE); attention pass B then finds its inputs ready.
        sa = [None] * NPAIR
        sb_ = [None] * NPAIR
        sa[0] = phase1a(0, xts[0])
        sb_[0] = phase1b(0, sa[0])
        for p in range(1, NPAIR):
            if p + 5 < NPAIR:
                xts[p + 5] = prefetch(p + 5)
            sa[p] = phase1a(p, xts[p])
            sb_[p] = phase1b(p, sa[p])
            phase2(sb_[p - 1])
        phase2(sb_[NPAIR - 1])

    nc.compile()
    return nc


def _host_prep(inputs):
    f = np.float32
    import ml_dtypes as _md

    _bf = _md.bfloat16
    x = np.ascontiguousarray(np.asarray(inputs["x"], dtype=f).astype(_bf))
    wq_full = np.ascontiguousarray(
        (np.asarray(inputs["wq"], dtype=f).transpose(1, 0, 2).reshape(C, C)
         * (C ** -0.5)).astype(_bf)
    )
    wk_full = np.ascontiguousarray(
        np.asarray(inputs["wk"], dtype=f).transpose(1, 0, 2)
        .reshape(C, C).astype(_bf)
    )
    wv_full = np.ascontiguousarray(
        np.asarray(inputs["wv"], dtype=f).transpose(1, 0, 2)
        .reshape(C, C).astype(_bf)
    )
    import ml_dtypes

    bf = ml_dtypes.bfloat16
    wp = np.ascontiguousarray(np.asarray(inputs["w_proj"], dtype=f).astype(bf))
    wqkvp = np.ascontiguousarray(
        np.concatenate([wq_full, wk_full, wv_full, wp], axis=1))
    w1 = np.ascontiguousarray(np.asarray(inputs["w1"], dtype=f).astype(bf))
    w2 = np.ascontiguousarray(np.asarray(inputs["w2"], dtype=f).astype(bf))
    tile128 = lambda v: np.ascontiguousarray(
        np.broadcast_to(np.asarray(v, dtype=f), (128, C))
    )
    g1 = tile128(inputs["ln1_g"])
    b1ln = tile128(inputs["ln1_b"])
    g2 = tile128(inputs["ln2_g"])
    b2ln = tile128(inputs["ln2_b"])
    bpb = tile128(inputs["b_proj"])
    b2b = tile128(inputs["b2"])
    b1c = np.ascontiguousarray(
        np.asarray(inputs["b1"], dtype=f).reshape(KH, 128).T)
    # transposed-scores mask bias [k, q]: 0 where valid (k<=q), -30000
    # where masked; exp(s - 30000) == 0 exactly in bf16.
    triu = np.triu(np.ones((128, 128), dtype=np.float32))
    maskT = np.concatenate(
        [triu, np.ones((128, 128), dtype=np.float32), triu], axis=1)
    maskbias = (maskT - 1.0) * 30000.0
    identb = np.eye(128, dtype=f)
    consts = np.ascontiguousarray(
        np.concatenate([maskbias, identb], axis=1).astype(bf))

    flags = (
        bool(not np.all(np.asarray(inputs["ln1_g"]) == 1.0)),
        bool(np.any(np.asarray(inputs["ln1_b"]))),
        bool(not np.all(np.asarray(inputs["ln2_g"]) == 1.0)),
        bool(np.any(np.asarray(inputs["ln2_b"]))),
        bool(np.any(np.asarray(inputs["b_proj"]))),
        bool(np.any(np.asarray(inputs["b1"]))),
        bool(np.any(np.asarray(inputs["b2"]))),
    )
    shared = dict(
        wqkvp=wqkvp, w1=w1, w2=w2,
        g1=g1, b1ln=b1ln, g2=g2, b2ln=b2ln, bpb=bpb, b2b=b2b, b1c=b1c,
        consts=consts,
    )
    in_maps = []
    for i in range(NCORES):
        m = dict(shared)
        m["x"] = np.ascontiguousarray(x[i * BL : (i + 1) * BL])
        in_maps.append(m)
    return in_maps, flags


_NC_CACHE = {}


def _get_program(flags):
    key = (flags, _STAGE)
    if key not in _NC_CACHE:
        _NC_CACHE[key] = build_program(*flags)
    return _NC_CACHE[key]


def run(inputs, **spmd_kwargs):
    from concourse.bass_utils import run_bass_kernel_spmd

    in_maps, flags = _host_prep(inputs)
    nc = _get_program(flags)
    bkr = run_bass_kernel_spmd(nc, in_maps, list(range(NCORES)), **spmd_kwargs)
    outs = [bkr.results[i]["out"] for i in range(NCORES)]
    return np.concatenate(outs, axis=0).astype(np.float32), bkr


def kernel(**inputs):
    full, _ = run(inputs)
    return full

